# revision 6
# baseline (speedup 1.0000x reference)
"""CIN (Compressed Interaction Network) kernel for Trainium2, 8 NeuronCores.

Reference computation (per sample b, NFIELD=64, NEMB=64, NFILTER=128, 3 layers):
    xk_{l+1}[o, e] = relu( sum_{f,c} W_l[o, f*C+c] * x0[f, e] * xk_l[c, e] )
    pooled_l = sum_e xk_{l+1};  y = concat(pooled) @ Wa.T

Strategy:
  - Data-parallel over batch: 32 samples/core, free axis J = 32*64 = 2048 (b-major,
    e-minor). Columns are independent through all layers; only the final pooled
    sum groups by b.
  - Per layer the GEMM is out = W @ H with H[(f,c), j] = x0[f,j] * xk[c,j]
    (Khatri-Rao column structure). H is materialized K-tile by K-tile in bf16 by
    DVE/GPSIMD tensor_tensor; the x0_f modulator rows are partition-replicated
    via DMA from DRAM (stride-0 source AP). PE runs bf16 matmuls (N=512)
    accumulating in PSUM; ScalarE applies ReLU into the next layer's bf16 xk;
    VectorE reduces the pooled sums in fp32.
  - Weights are host-pre-transposed to (K, O) bf16 so K-tiles DMA straight in.
"""

import sys

if "/opt/trn_rl_repo" not in sys.path:
    sys.path.insert(0, "/opt/trn_rl_repo")

import numpy as np
import ml_dtypes

B, F, E, O = 256, 64, 64, 128
NCORES = 8
BC = B // NCORES          # samples per core
J = BC * E                # free columns per core
JB = 512                  # free-block size (one PSUM bank)
NJ = J // JB              # 4 free blocks
KT = [32, 64, 64]         # K-tiles (of 128) per layer

_BF16 = ml_dtypes.bfloat16
_STATE = {}


def _build_nc():
    import concourse.bass as bass
    import concourse.tile as tile
    import concourse.mybir as mybir
    from concourse import bacc

    dt = mybir.dt
    nc = bacc.Bacc("TRN2", target_bir_lowering=False, debug=False)

    x0b = nc.dram_tensor("x0b", [F, J], dt.bfloat16, kind="ExternalInput")
    x0dup = nc.dram_tensor("x0dup", [128, J], dt.bfloat16, kind="ExternalInput")
    w0t = nc.dram_tensor("w0t", [F * F, O], dt.bfloat16, kind="ExternalInput")
    w1t = nc.dram_tensor("w1t", [F * O, O], dt.bfloat16, kind="ExternalInput")
    w2t = nc.dram_tensor("w2t", [F * O, O], dt.bfloat16, kind="ExternalInput")
    wa = nc.dram_tensor("wa", [O, 3], dt.float32, kind="ExternalInput")
    y = nc.dram_tensor("y", [1, BC], dt.float32, kind="ExternalOutput")

    with tile.TileContext(nc) as tc:
        with (
            tc.tile_pool(name="wpool", bufs=1) as wpool,
            tc.tile_pool(name="xpool", bufs=1) as xpool,
            tc.tile_pool(name="modpool", bufs=18) as modpool,
            tc.tile_pool(name="mod0pool", bufs=5) as mod0pool,
            tc.tile_pool(name="hpool", bufs=6) as hpool,
            tc.tile_pool(name="h0pool", bufs=4) as h0pool,
            tc.tile_pool(name="xkpool", bufs=3) as xkpool,
            tc.tile_pool(name="psum", bufs=2, space="PSUM") as psum_pool,
            tc.tile_pool(name="psumy", bufs=1, space="PSUM") as psumy_pool,
        ):
            # --- static loads -------------------------------------------------
            x0dup_sb = xpool.tile([128, J], dt.bfloat16, tag="x0dup")
            nc.sync.dma_start(x0dup_sb[:], x0dup[:])
            wa_sb = xpool.tile([O, 3], dt.float32, tag="wa")
            nc.sync.dma_start(wa_sb[:], wa[:])
            w_sb = []
            for li, (wd, kt) in enumerate(zip((w0t, w1t, w2t), KT)):
                w = wpool.tile([128, kt, O], dt.bfloat16, tag=f"w{li}")
                nc.sync.dma_start(w[:], wd[:].rearrange("(t p) o -> p t o", p=128))
                w_sb.append(w)
            pooled = [xpool.tile([O, BC], dt.float32, tag=f"pooled{l}", name=f"pooled{l}") for l in range(3)]

            # --- main loop over free blocks ----------------------------------
            for jj in range(NJ):
                jsl = slice(JB * jj, JB * (jj + 1))
                # Modulator quad tiles: mods[q][p, i, :] = x0[4q+i, jsl] for
                # every partition p. Seed partition 0 from DRAM (4 KB), then
                # log-double partitions via SBUF->SBUF DMA to avoid the 128x
                # HBM read amplification of a stride-0 DRAM broadcast.
                mods = []
                mod0s = []
                for q in range(16):
                    m = modpool.tile([128, 4, JB], dt.bfloat16, tag="mod", name=f"mod{jj}_{q}")
                    nc.sync.dma_start(
                        m[0:1, :, :],
                        x0b[4 * q : 4 * q + 4, jsl].rearrange("(x f) e -> x f e", x=1),
                    )
                    p = 1
                    while p < 128:
                        nc.sync.dma_start(m[p : 2 * p, :, :], m[0:p, :, :])
                        p *= 2
                    mods.append(m)
                    # L0-layout tile: m0[p, i, :] = x0[4q + 2i + (p >= 64), jsl]
                    m0 = mod0pool.tile([128, 2, JB], dt.bfloat16, tag="mod0", name=f"mod0{jj}_{q}")
                    nc.sync.dma_start(m0[0:64, :, :], m[0:64, 0::2, :])
                    nc.sync.dma_start(m0[64:128, :, :], m[64:128, 1::2, :])
                    mod0s.append(m0)

                xk = x0dup_sb[:, jsl]  # layer-0 "xk" input (c = 0..63, duplicated)
                for l in range(3):
                    kt = KT[l]
                    acc = psum_pool.tile([128, JB], dt.float32, tag="acc", name=f"acc{jj}_{l}")
                    if l == 0:
                        # K-tile t rows: p<64 -> k=(f=2t, c=p); p>=64 -> (f=2t+1, c=p-64)
                        in0 = (
                            x0dup_sb[:, jsl]
                            .rearrange("p (x n) -> p x n", x=1)
                            .to_broadcast((128, 2, JB))
                        )
                        for q in range(16):
                            h0 = h0pool.tile([128, 2, JB], dt.bfloat16, tag="h0", name=f"h0_{jj}_{q}")
                            eng = nc.gpsimd if q in (5, 11) else nc.vector
                            eng.tensor_tensor(
                                h0[:], in0, mod0s[q][:], op=mybir.AluOpType.mult
                            )
                            for i in range(2):
                                t = 2 * q + i
                                nc.tensor.matmul(
                                    acc[:], w_sb[0][:, t, :], h0[:, i, :],
                                    start=(t == 0), stop=(t == kt - 1),
                                )
                    else:
                        in0 = (
                            xk.rearrange("p (x n) -> p x n", x=1)
                            .to_broadcast((128, 4, JB))
                        )
                        for q in range(16):
                            h = hpool.tile([128, 4, JB], dt.bfloat16, tag="h", name=f"h{jj}_{l}_{q}")
                            eng = nc.gpsimd if (q % 4 == 2) else nc.vector
                            eng.tensor_tensor(
                                h[:], in0, mods[q][:], op=mybir.AluOpType.mult
                            )
                            for i in range(4):
                                t = 4 * q + i
                                nc.tensor.matmul(
                                    acc[:], w_sb[l][:, t, :], h[:, i, :],
                                    start=(t == 0), stop=(t == kt - 1),
                                )
                    # epilogue: relu -> bf16 xk for next layer; pooled sums (fp32)
                    xk_new = xkpool.tile([128, JB], dt.bfloat16, tag="xk")
                    nc.scalar.activation(
                        xk_new[:], acc[:], mybir.ActivationFunctionType.Relu
                    )
                    nc.vector.tensor_reduce(
                        pooled[l][:, 8 * jj : 8 * jj + 8],
                        xk_new[:].rearrange("p (b e) -> p b e", e=E),
                        axis=mybir.AxisListType.X,
                        op=mybir.AluOpType.add,
                    )
                    xk = xk_new[:]

            # --- head: y[b] = sum_l wa[:, l] . pooled[l][:, b] ----------------
            yac = psumy_pool.tile([1, BC], dt.float32, tag="yac")
            for l in range(3):
                nc.tensor.matmul(
                    yac[:], wa_sb[:, l : l + 1], pooled[l][:],
                    start=(l == 0), stop=(l == 2),
                )
            y_sb = xpool.tile([1, BC], dt.float32, tag="ysb")
            nc.scalar.copy(y_sb[:], yac[:])
            nc.sync.dma_start(y[:], y_sb[:])

    nc.finalize()
    return nc


def _get_nc():
    if "nc" not in _STATE:
        _STATE["nc"] = _build_nc()
    return _STATE["nc"]


def _prep_in_maps(x, W0, W1, W2, Wa):
    x = np.asarray(x, dtype=np.float32)
    w0t = np.ascontiguousarray(np.asarray(W0, np.float32).T).astype(_BF16)
    w1t = np.ascontiguousarray(np.asarray(W1, np.float32).T).astype(_BF16)
    w2t = np.ascontiguousarray(np.asarray(W2, np.float32).T).astype(_BF16)
    wa = np.ascontiguousarray(np.asarray(Wa, np.float32).reshape(3, O).T)
    in_maps = []
    for c in range(NCORES):
        xc = x[c * BC : (c + 1) * BC]                       # (BC, F, E)
        x0 = np.ascontiguousarray(xc.transpose(1, 0, 2).reshape(F, J))
        x0b = x0.astype(_BF16)
        x0dup = np.concatenate([x0b, x0b], axis=0)          # (128, J)
        in_maps.append(
            {
                "x0b": x0b,
                "x0dup": x0dup,
                "w0t": w0t,
                "w1t": w1t,
                "w2t": w2t,
                "wa": wa,
            }
        )
    return in_maps


def _run(inputs, trace=False, **kwargs):
    from concourse.bass_utils import run_bass_kernel_spmd

    nc = _get_nc()
    in_maps = _prep_in_maps(**inputs)
    res = run_bass_kernel_spmd(
        nc, in_maps, core_ids=list(range(NCORES)), trace=trace, **kwargs
    )
    y = np.concatenate(
        [np.asarray(r["y"], np.float32).reshape(BC) for r in res.results]
    )
    return y, res


def kernel(**inputs) -> np.ndarray:
    y, _ = _run(inputs, trace=False)
    return y


# revision 9
# speedup vs baseline: 1.2362x; 1.2362x over previous
"""CIN (Compressed Interaction Network) kernel for Trainium2, 8 NeuronCores.

Reference computation (per sample b, NFIELD=64, NEMB=64, NFILTER=128, 3 layers):
    xk_{l+1}[o, e] = relu( sum_{f,c} W_l[o, f*C+c] * x0[f, e] * xk_l[c, e] )
    pooled_l = sum_e xk_{l+1};  y = concat(pooled) @ Wa.T

Strategy:
  - Data-parallel over batch: 32 samples/core, free axis J = 32*64 = 2048 (b-major,
    e-minor). Columns are independent through all layers; only the final pooled
    sum groups by b.
  - Per layer the GEMM is out = W @ H with H[(f,c), j] = x0[f,j] * xk[c,j]
    (Khatri-Rao column structure). H is materialized K-tile by K-tile in bf16 by
    DVE/GPSIMD tensor_tensor with plain 2D unit-stride APs (to hit the DVE
    2x_1P perf mode); the x0_f modulator rows are partition-replicated in
    "hex" tiles of 16 fields via one seed DMA from a host-side 8x-replicated
    copy plus log2 partition-doubling SBUF->SBUF DMAs. PE runs bf16 matmuls
    (N=512) accumulating in PSUM; ScalarE applies ReLU 4x into a repeated
    next-layer input xk4; VectorE reduces the pooled sums in fp32.
  - Weights are host-pre-transposed to (K, O) bf16 so K-tiles DMA straight in.
"""

import sys

if "/opt/trn_rl_repo" not in sys.path:
    sys.path.insert(0, "/opt/trn_rl_repo")

import numpy as np
import ml_dtypes

B, F, E, O = 256, 64, 64, 128
NCORES = 8
BC = B // NCORES          # samples per core
J = BC * E                # free columns per core
JB = 512                  # free-block size (one PSUM bank)
NJ = J // JB              # 4 free blocks
KT = [32, 64, 64]         # K-tiles (of 128) per layer

_BF16 = ml_dtypes.bfloat16
_STATE = {}


def _build_nc():
    import concourse.bass as bass
    import concourse.tile as tile
    import concourse.mybir as mybir
    from concourse import bacc

    dt = mybir.dt
    nc = bacc.Bacc("TRN2", target_bir_lowering=False, debug=False)

    # x0rep8: x0 with every field row replicated 8x -> (8*F, J)
    x0rep8 = nc.dram_tensor("x0rep8", [8 * F, J], dt.bfloat16, kind="ExternalInput")
    x0dup = nc.dram_tensor("x0dup", [128, J], dt.bfloat16, kind="ExternalInput")
    w0t = nc.dram_tensor("w0t", [F * F, O], dt.bfloat16, kind="ExternalInput")
    w1t = nc.dram_tensor("w1t", [F * O, O], dt.bfloat16, kind="ExternalInput")
    w2t = nc.dram_tensor("w2t", [F * O, O], dt.bfloat16, kind="ExternalInput")
    wa = nc.dram_tensor("wa", [O, 3], dt.float32, kind="ExternalInput")
    y = nc.dram_tensor("y", [1, BC], dt.float32, kind="ExternalOutput")

    HEXW = 16 * JB            # free width of a 16-field modulator tile
    M0W = 8 * JB              # free width of the L0-layout modulator tile

    with tile.TileContext(nc) as tc:
        with (
            tc.tile_pool(name="wpool", bufs=1) as wpool,
            tc.tile_pool(name="xpool", bufs=1) as xpool,
            tc.tile_pool(name="modpool", bufs=5) as modpool,
            tc.tile_pool(name="mod0pool", bufs=3) as mod0pool,
            tc.tile_pool(name="hpool", bufs=4) as hpool,
            tc.tile_pool(name="h0pool", bufs=3) as h0pool,
            tc.tile_pool(name="xkpool", bufs=2) as xkpool,
            tc.tile_pool(name="x0pool", bufs=2) as x0pool,
            tc.tile_pool(name="psum", bufs=2, space="PSUM") as psum_pool,
            tc.tile_pool(name="psumy", bufs=1, space="PSUM") as psumy_pool,
        ):
            # --- static loads -------------------------------------------------
            wa_sb = xpool.tile([O, 3], dt.float32, tag="wa")
            nc.sync.dma_start(wa_sb[:], wa[:])
            w_sb = []
            for li, (wd, kt) in enumerate(zip((w0t, w1t, w2t), KT)):
                w = wpool.tile([128, kt, O], dt.bfloat16, tag=f"w{li}", name=f"w{li}")
                nc.sync.dma_start(w[:], wd[:].rearrange("(t p) o -> p t o", p=128))
                w_sb.append(w)
            pooled = [
                xpool.tile([O, BC], dt.float32, tag=f"pooled{l}", name=f"pooled{l}")
                for l in range(3)
            ]

            tt_rr = [0]  # round-robin counter for DVE/GPSIMD routing

            def tt_engine(gpsimd_every=4):
                tt_rr[0] += 1
                return nc.gpsimd if (tt_rr[0] % gpsimd_every == 0) else nc.vector

            # --- main loop over free blocks ----------------------------------
            for jj in range(NJ):
                jsl = slice(JB * jj, JB * (jj + 1))
                # Modulator hex tiles: mh[p, 512*i + e] = x0[16*hx + i, jsl][e]
                # for all p. Seed partitions 0:8 from x0rep8, then partition-
                # double via SBUF->SBUF DMA (8->16->32->64->128).
                mhs, m0s = [], []
                for hx in range(4):
                    mh = modpool.tile([128, HEXW], dt.bfloat16, tag="mod", name=f"mh{jj}_{hx}")
                    seed = x0rep8[8 * 16 * hx : 8 * 16 * hx + 128, jsl].rearrange(
                        "(i p) e -> p i e", p=8
                    )
                    nc.sync.dma_start(
                        mh[0:8, :].rearrange("p (i e) -> p i e", e=JB), seed
                    )
                    p = 8
                    while p < 128:
                        nc.sync.dma_start(mh[p : 2 * p, :], mh[0:p, :])
                        p *= 2
                    mhs.append(mh)
                    # L0 layout: m0[p, 512*a + e] = x0[16*hx + 2a + (p>=64), ...]
                    m0 = mod0pool.tile([128, M0W], dt.bfloat16, tag="mod0", name=f"m0{jj}_{hx}")
                    mh3 = mh[:].rearrange("p (i e) -> p i e", e=JB)
                    nc.sync.dma_start(
                        m0[0:64, :].rearrange("p (a e) -> p a e", e=JB),
                        mh3[0:64, 0::2, :],
                    )
                    nc.sync.dma_start(
                        m0[64:128, :].rearrange("p (a e) -> p a e", e=JB),
                        mh3[64:128, 1::2, :],
                    )
                    m0s.append(m0)

                # x04: x0dup[:, jsl] repeated twice -> (128, 1024), via one DMA
                x04 = x0pool.tile([128, 2 * JB], dt.bfloat16, tag="x04", name=f"x04_{jj}")
                nc.sync.dma_start(
                    x04[:].rearrange("p (i e) -> p i e", e=JB),
                    x0dup[:, jsl]
                    .rearrange("p (i e) -> p i e", i=1)
                    .to_broadcast((128, 2, JB)),
                )

                xk4 = None
                for l in range(3):
                    kt = KT[l]
                    acc = psum_pool.tile([128, JB], dt.float32, tag="acc", name=f"acc{jj}_{l}")
                    if l == 0:
                        # pair ops: hex hx, s in 0..3 -> K-tiles t = 8*hx+2s, +1
                        for hx in range(4):
                            for s in range(4):
                                h0 = h0pool.tile([128, 2 * JB], dt.bfloat16, tag="h0", name=f"h0_{jj}_{hx}_{s}")
                                eng = tt_engine()
                                eng.tensor_tensor(
                                    h0[:], x04[:],
                                    m0s[hx][:, 2 * JB * s : 2 * JB * (s + 1)],
                                    op=mybir.AluOpType.mult,
                                )
                                for i in range(2):
                                    t = 8 * hx + 2 * s + i
                                    nc.tensor.matmul(
                                        acc[:], w_sb[0][:, t, :],
                                        h0[:, JB * i : JB * (i + 1)],
                                        start=(t == 0), stop=(t == kt - 1),
                                    )
                    else:
                        for hx in range(4):
                            for s in range(4):
                                h = hpool.tile([128, 4 * JB], dt.bfloat16, tag="h", name=f"h{jj}_{l}_{hx}_{s}")
                                eng = tt_engine()
                                eng.tensor_tensor(
                                    h[:], xk4[:],
                                    mhs[hx][:, 4 * JB * s : 4 * JB * (s + 1)],
                                    op=mybir.AluOpType.mult,
                                )
                                for i in range(4):
                                    t = 16 * hx + 4 * s + i
                                    nc.tensor.matmul(
                                        acc[:], w_sb[l][:, t, :],
                                        h[:, JB * i : JB * (i + 1)],
                                        start=(t == 0), stop=(t == kt - 1),
                                    )
                    # epilogue: relu 4x into xk4 (repeated next-layer input)
                    xk4_new = xkpool.tile([128, 4 * JB], dt.bfloat16, tag="xk4", name=f"xk4_{jj}_{l}")
                    for i in range(4):
                        nc.scalar.activation(
                            xk4_new[:, JB * i : JB * (i + 1)], acc[:],
                            mybir.ActivationFunctionType.Relu,
                        )
                    nc.vector.tensor_reduce(
                        pooled[l][:, 8 * jj : 8 * jj + 8],
                        xk4_new[:, 0:JB].rearrange("p (b e) -> p b e", e=E),
                        axis=mybir.AxisListType.X,
                        op=mybir.AluOpType.add,
                    )
                    xk4 = xk4_new

            # --- head: y[b] = sum_l wa[:, l] . pooled[l][:, b] ----------------
            yac = psumy_pool.tile([1, BC], dt.float32, tag="yac")
            for l in range(3):
                nc.tensor.matmul(
                    yac[:], wa_sb[:, l : l + 1], pooled[l][:],
                    start=(l == 0), stop=(l == 2),
                )
            y_sb = xpool.tile([1, BC], dt.float32, tag="ysb")
            nc.scalar.copy(y_sb[:], yac[:])
            nc.sync.dma_start(y[:], y_sb[:])

    nc.finalize()
    return nc


def _get_nc():
    if "nc" not in _STATE:
        _STATE["nc"] = _build_nc()
    return _STATE["nc"]


def _prep_in_maps(x, W0, W1, W2, Wa):
    x = np.asarray(x, dtype=np.float32)
    w0t = np.ascontiguousarray(np.asarray(W0, np.float32).T).astype(_BF16)
    w1t = np.ascontiguousarray(np.asarray(W1, np.float32).T).astype(_BF16)
    w2t = np.ascontiguousarray(np.asarray(W2, np.float32).T).astype(_BF16)
    wa = np.ascontiguousarray(np.asarray(Wa, np.float32).reshape(3, O).T)
    in_maps = []
    for c in range(NCORES):
        xc = x[c * BC : (c + 1) * BC]                       # (BC, F, E)
        x0 = np.ascontiguousarray(xc.transpose(1, 0, 2).reshape(F, J))
        x0b = x0.astype(_BF16)
        x0rep8 = np.repeat(x0b, 8, axis=0)                  # (512, J)
        x0dup = np.concatenate([x0b, x0b], axis=0)          # (128, J)
        in_maps.append(
            {
                "x0rep8": x0rep8,
                "x0dup": x0dup,
                "w0t": w0t,
                "w1t": w1t,
                "w2t": w2t,
                "wa": wa,
            }
        )
    return in_maps


def _run(inputs, trace=False, **kwargs):
    from concourse.bass_utils import run_bass_kernel_spmd

    nc = _get_nc()
    in_maps = _prep_in_maps(**inputs)
    res = run_bass_kernel_spmd(
        nc, in_maps, core_ids=list(range(NCORES)), trace=trace, **kwargs
    )
    y = np.concatenate(
        [np.asarray(r["y"], np.float32).reshape(BC) for r in res.results]
    )
    return y, res


def kernel(**inputs) -> np.ndarray:
    y, _ = _run(inputs, trace=False)
    return y


# revision 11
# speedup vs baseline: 2.3177x; 1.8749x over previous
"""CIN (Compressed Interaction Network) kernel for Trainium2, 8 NeuronCores.

Reference computation (per sample b, NFIELD=64, NEMB=64, NFILTER=128, 3 layers):
    xk_{l+1}[o, e] = relu( sum_{f,c} W_l[o, f*C+c] * x0[f, e] * xk_l[c, e] )
    pooled_l = sum_e xk_{l+1};  y = concat(pooled) @ Wa.T

Strategy:
  - Data-parallel over batch: 32 samples/core, free axis J = 32*64 = 2048 (b-major,
    e-minor). Columns are independent through all layers; only the final pooled
    sum groups by b.
  - Per layer the GEMM is out = W @ H with H[(f,c), j] = x0[f,j] * xk[c,j]
    (Khatri-Rao column structure). H is materialized K-tile by K-tile in bf16 by
    DVE tensor_tensor with plain 2D unit-stride APs (DVE 2x_1P perf mode).
  - Layer 0 is symmetric (xk = x0): W0 is host-folded onto upper-triangle
    (f<=c) pairs, K = 2080 -> 17 K-tiles (vs 32), and both TT operands are
    host-gathered arrays (x0pack_f/x0pack_c) loaded straight from DRAM.
  - Layers 1-2: the x0_f modulator rows are partition-replicated in "hex"
    tiles of 16 fields: one seed DMA (32 partitions, from a host-side
    32x-replicated x0rep32) + 2 partition-doubling SBUF->SBUF DMAs.
    DMA issue is wave-interleaved across hex tiles to avoid FIFO
    head-of-line blocking on the Sync queue.
  - PE runs bf16 matmuls (N=512) accumulating in PSUM; ScalarE applies ReLU
    4x into a repeated next-layer input xk4; VectorE reduces pooled in fp32.
  - Weights host-pre-transposed to (K, O) bf16; W1/W2 loaded via the GPSIMD
    DMA queue so they don't block the Sync queue at startup.
"""

import sys

if "/opt/trn_rl_repo" not in sys.path:
    sys.path.insert(0, "/opt/trn_rl_repo")

import numpy as np
import ml_dtypes

B, F, E, O = 256, 64, 64, 128
NCORES = 8
BC = B // NCORES          # samples per core
J = BC * E                # free columns per core
JB = 512                  # free-block size (one PSUM bank)
NJ = J // JB              # 4 free blocks
KT0 = 17                  # layer-0 K-tiles (packed symmetric, 2176 = 17*128)
K0 = KT0 * 128
KT = [KT0, 64, 64]

_BF16 = ml_dtypes.bfloat16
_STATE = {}

# layer-0 packed pair enumeration (f <= c), padded to K0 with (0, 0)
_PAIRS = [(f, c) for f in range(F) for c in range(f, F)]
_F_IDX = np.array([p[0] for p in _PAIRS] + [0] * (K0 - len(_PAIRS)), np.int64)
_C_IDX = np.array([p[1] for p in _PAIRS] + [0] * (K0 - len(_PAIRS)), np.int64)


def _build_nc():
    import concourse.bass as bass
    import concourse.tile as tile
    import concourse.mybir as mybir
    from concourse import bacc

    dt = mybir.dt
    nc = bacc.Bacc("TRN2", target_bir_lowering=False, debug=False)

    x0rep32 = nc.dram_tensor("x0rep32", [32 * F, J], dt.bfloat16, kind="ExternalInput")
    x0packf = nc.dram_tensor("x0packf", [K0, J], dt.bfloat16, kind="ExternalInput")
    x0packc = nc.dram_tensor("x0packc", [K0, J], dt.bfloat16, kind="ExternalInput")
    w0t = nc.dram_tensor("w0t", [K0, O], dt.bfloat16, kind="ExternalInput")
    w1t = nc.dram_tensor("w1t", [F * O, O], dt.bfloat16, kind="ExternalInput")
    w2t = nc.dram_tensor("w2t", [F * O, O], dt.bfloat16, kind="ExternalInput")
    wa = nc.dram_tensor("wa", [O, 3], dt.float32, kind="ExternalInput")
    y = nc.dram_tensor("y", [1, BC], dt.float32, kind="ExternalOutput")

    HEXW = 16 * JB            # free width of a 16-field modulator tile
    PKW = KT0 * JB            # free width of a packed layer-0 operand tile

    with tile.TileContext(nc) as tc:
        with (
            tc.tile_pool(name="wpool", bufs=1) as wpool,
            tc.tile_pool(name="xpool", bufs=1) as xpool,
            tc.tile_pool(name="modpool", bufs=4) as modpool,
            tc.tile_pool(name="packpool", bufs=1) as packpool,
            tc.tile_pool(name="hpool", bufs=4) as hpool,
            tc.tile_pool(name="xkpool", bufs=2) as xkpool,
            tc.tile_pool(name="psum", bufs=2, space="PSUM") as psum_pool,
            tc.tile_pool(name="psumy", bufs=1, space="PSUM") as psumy_pool,
        ):
            # --- static loads -------------------------------------------------
            wa_sb = xpool.tile([O, 3], dt.float32, tag="wa")
            nc.sync.dma_start(wa_sb[:], wa[:])
            w_sb = []
            for li, (wd, kt) in enumerate(zip((w0t, w1t, w2t), KT)):
                w = wpool.tile([128, kt, O], dt.bfloat16, tag=f"w{li}", name=f"w{li}")
                eng = nc.sync if li == 0 else nc.gpsimd
                eng.dma_start(w[:], wd[:].rearrange("(t p) o -> p t o", p=128))
                w_sb.append(w)
            pooled = [
                xpool.tile([O, BC], dt.float32, tag=f"pooled{l}", name=f"pooled{l}")
                for l in range(3)
            ]

            # --- main loop over free blocks ----------------------------------
            for jj in range(NJ):
                jsl = slice(JB * jj, JB * (jj + 1))
                # layer-0 packed operand tiles (straight DRAM gather loads)
                p0f = packpool.tile([128, PKW], dt.bfloat16, tag="p0f", name=f"p0f{jj}")
                p0c = packpool.tile([128, PKW], dt.bfloat16, tag="p0c", name=f"p0c{jj}")
                nc.sync.dma_start(
                    p0f[:].rearrange("p (t e) -> p t e", e=JB),
                    x0packf[:, jsl].rearrange("(t p) e -> p t e", p=128),
                )
                nc.sync.dma_start(
                    p0c[:].rearrange("p (t e) -> p t e", e=JB),
                    x0packc[:, jsl].rearrange("(t p) e -> p t e", p=128),
                )
                # modulator hex tiles for layers 1-2:
                #   mh[p, 512*i + e] = x0[16*hx + i, jsl][e] for every p
                # wave-interleaved issue: seeds for all hexes, then doubling waves
                mhs = []
                for hx in range(4):
                    mh = modpool.tile(
                        [128, HEXW], dt.bfloat16, tag="mod", name=f"mh{jj}_{hx}"
                    )
                    seed = x0rep32[32 * 16 * hx : 32 * 16 * hx + 512, jsl].rearrange(
                        "(i p) e -> p i e", p=32
                    )
                    nc.sync.dma_start(
                        mh[0:32, :].rearrange("p (i e) -> p i e", e=JB), seed
                    )
                    mhs.append(mh)
                for hx in range(4):
                    nc.sync.dma_start(mhs[hx][32:64, :], mhs[hx][0:32, :])
                for hx in range(4):
                    nc.sync.dma_start(mhs[hx][64:128, :], mhs[hx][0:64, :])

                xk4 = None
                for l in range(3):
                    kt = KT[l]
                    acc = psum_pool.tile(
                        [128, JB], dt.float32, tag="acc", name=f"acc{jj}_{l}"
                    )
                    if l == 0:
                        # 8 pair ops + 1 single op over 17 packed K-tiles
                        for s in range(9):
                            nk = 2 if s < 8 else 1
                            h = hpool.tile(
                                [128, 4 * JB], dt.bfloat16, tag="h", name=f"h0_{jj}_{s}"
                            )
                            w_ = JB * nk
                            nc.vector.tensor_tensor(
                                h[:, 0:w_],
                                p0c[:, 2 * JB * s : 2 * JB * s + w_],
                                p0f[:, 2 * JB * s : 2 * JB * s + w_],
                                op=mybir.AluOpType.mult,
                            )
                            for i in range(nk):
                                t = 2 * s + i
                                nc.tensor.matmul(
                                    acc[:], w_sb[0][:, t, :],
                                    h[:, JB * i : JB * (i + 1)],
                                    start=(t == 0), stop=(t == kt - 1),
                                )
                    else:
                        for hx in range(4):
                            for s in range(4):
                                h = hpool.tile(
                                    [128, 4 * JB], dt.bfloat16, tag="h",
                                    name=f"h{jj}_{l}_{hx}_{s}",
                                )
                                nc.vector.tensor_tensor(
                                    h[:], xk4[:],
                                    mhs[hx][:, 4 * JB * s : 4 * JB * (s + 1)],
                                    op=mybir.AluOpType.mult,
                                )
                                for i in range(4):
                                    t = 16 * hx + 4 * s + i
                                    nc.tensor.matmul(
                                        acc[:], w_sb[l][:, t, :],
                                        h[:, JB * i : JB * (i + 1)],
                                        start=(t == 0), stop=(t == kt - 1),
                                    )
                    # epilogue: relu 4x into xk4 (repeated next-layer input)
                    xk4_new = xkpool.tile(
                        [128, 4 * JB], dt.bfloat16, tag="xk4", name=f"xk4_{jj}_{l}"
                    )
                    for i in range(4):
                        nc.scalar.activation(
                            xk4_new[:, JB * i : JB * (i + 1)], acc[:],
                            mybir.ActivationFunctionType.Relu,
                        )
                    nc.vector.tensor_reduce(
                        pooled[l][:, 8 * jj : 8 * jj + 8],
                        xk4_new[:, 0:JB].rearrange("p (b e) -> p b e", e=E),
                        axis=mybir.AxisListType.X,
                        op=mybir.AluOpType.add,
                    )
                    xk4 = xk4_new

            # --- head: y[b] = sum_l wa[:, l] . pooled[l][:, b] ----------------
            yac = psumy_pool.tile([1, BC], dt.float32, tag="yac")
            for l in range(3):
                nc.tensor.matmul(
                    yac[:], wa_sb[:, l : l + 1], pooled[l][:],
                    start=(l == 0), stop=(l == 2),
                )
            y_sb = xpool.tile([1, BC], dt.float32, tag="ysb")
            nc.scalar.copy(y_sb[:], yac[:])
            nc.sync.dma_start(y[:], y_sb[:])

    nc.finalize()
    return nc


def _get_nc():
    if "nc" not in _STATE:
        _STATE["nc"] = _build_nc()
    return _STATE["nc"]


def _pack_w0(W0):
    # fold symmetric (f, c) weight pairs onto f <= c; pad to K0 with zeros
    w = np.asarray(W0, np.float32).reshape(O, F, F)
    wp = np.zeros((O, K0), np.float32)
    k = 0
    for f in range(F):
        wp[:, k] = w[:, f, f]
        k += 1
        n = F - f - 1
        if n:
            wp[:, k : k + n] = w[:, f, f + 1 :] + w[:, f + 1 :, f]
            k += n
    return wp


def _prep_in_maps(x, W0, W1, W2, Wa):
    x = np.asarray(x, dtype=np.float32)
    w0t = np.ascontiguousarray(_pack_w0(W0).T).astype(_BF16)
    w1t = np.ascontiguousarray(np.asarray(W1, np.float32).T).astype(_BF16)
    w2t = np.ascontiguousarray(np.asarray(W2, np.float32).T).astype(_BF16)
    wa = np.ascontiguousarray(np.asarray(Wa, np.float32).reshape(3, O).T)
    in_maps = []
    for c in range(NCORES):
        xc = x[c * BC : (c + 1) * BC]                       # (BC, F, E)
        x0 = np.ascontiguousarray(xc.transpose(1, 0, 2).reshape(F, J))
        x0b = x0.astype(_BF16)
        in_maps.append(
            {
                "x0rep32": np.repeat(x0b, 32, axis=0),
                "x0packf": np.ascontiguousarray(x0b[_F_IDX]),
                "x0packc": np.ascontiguousarray(x0b[_C_IDX]),
                "w0t": w0t,
                "w1t": w1t,
                "w2t": w2t,
                "wa": wa,
            }
        )
    return in_maps


def _run(inputs, trace=False, **kwargs):
    from concourse.bass_utils import run_bass_kernel_spmd

    nc = _get_nc()
    in_maps = _prep_in_maps(**inputs)
    res = run_bass_kernel_spmd(
        nc, in_maps, core_ids=list(range(NCORES)), trace=trace, **kwargs
    )
    y = np.concatenate(
        [np.asarray(r["y"], np.float32).reshape(BC) for r in res.results]
    )
    return y, res


def kernel(**inputs) -> np.ndarray:
    y, _ = _run(inputs, trace=False)
    return y
